# revision 4
# baseline (speedup 1.0000x reference)
"""Trainium2 Bass kernel for nn_ConvFilter (geometric-series conv filter).

Math (per batch b, output position l, feature f):
    t[o,l]  = sum_{i,k} conv_w[o,i,k] * x[l+k,i]          (valid conv, L=S-K+1)
    tau     = sigmoid(t + bias)
    out     = (sum_i tau^(7-i) * x[l+i,f]) / (sum_i tau^i)

Implementation:
  * transposed layout [feature, seq] on device; host pre/post-transposes.
  * everything fp16 on the wire: x and conv_w are converted to fp16 on the
    host (PE runs fp16 at full rate), output returns as fp16 and the host
    upcasts to fp32.  Validated numerically: absmax/scale ~1.3e-3.
  * conv: 16 accumulating fp16 matmuls per 512-wide l-tile;
    two overlapping l-tiles (0 and L-512) per feature block.
  * numerator in fp16 on DVE (2x packed mode):
        q_j = tau*x_{2j} + x_{2j+1}
        N   = (q0*T2 + q1)*T4 + (q2*T2 + q3),   T2 = tau^2, T4 = tau^4
    odd-shift windows read a one-element-shifted fp16 copy of x so every
    window stays 4-byte aligned (keeps the DVE 2x mode).
  * denominator fp32: D = (1+tau)(1+tau^2)(1+tau^4) as one custom DVE op;
    final out = N/D as ONE fused custom op (bitwise-NOT reciprocal seed +
    one Newton step + multiply by N, 6/8 stages, rel err ~2e-3), fp16 out.
  * engine split: ACT does sigmoid/squares/shifted-copy, DVE the chain,
    PE only matmuls.
  * data-parallel over batch: 8 batches/core on 8 cores, weights replicated.
"""

import numpy as np
from contextlib import ExitStack

import concourse.bass as bass
import concourse.tile as tile
from concourse import bacc, mybir
from concourse.bass_utils import run_bass_kernel_spmd
from concourse import dve_ops
from concourse.dve_ops import DveOp
from concourse.dve_spec import (
    Spec, Src0, Src1, Bin, AluOp, lower, sq, One, C0, C1, _has_src1,
)
from concourse.dve_uop import DveOpSpec

B, S, F, K = 64, 1024, 256, 8
L = S - K + 1  # 1017
NCORES = 8
BPC = B // NCORES
P = 128
NFB = F // P  # 2 feature blocks
LT = 512      # matmul l-tile width (one PSUM bank)
LE = L + 1    # even fp16 elementwise width (DVE 2x mode needs even counts)

# 1-NR reciprocal seed constants (minimax over x*bitcast(~x) in [-4.5,-4]).
RM_C0 = -0.23549792
RM_C1 = 2.0017324


def _register_op(name, spec, subdim=False):
    for existing in dve_ops.OPS:
        if existing.name == name:
            return existing
    shas = {}
    for ver in ("v3", "v4"):
        tmp = DveOpSpec(name=name, opcode=0, uops=lower(spec, ver=ver),
                        rd1_en=_has_src1(spec))
        shas[ver] = tmp.sha(ver)
    op = DveOp(name, spec, subdim=subdim, uops_sha=shas)
    dve_ops.OPS.append(op)
    dve_ops.CUSTOM_DVE_SPECS[name] = spec
    dve_ops._SUB_OPCODE_FOR_NAME[name] = (
        dve_ops._CUSTOM_DVE_ROW_BASE + len(dve_ops.OPS) - 1
    )
    assert dve_ops._SUB_OPCODE_FOR_NAME[name] < 0x20
    return op


def _get_ops():
    _t2 = sq(Src0)
    _t4 = sq(_t2)
    denom_spec = Spec(
        body=(Src0 + One) * (_t2 + One) * (_t4 + One),
        reference=lambda in0, in1, s0, s1, imm2: (
            (1.0 + in0) * (1.0 + in0 * in0) * (1.0 + in0 ** 4)
        ).astype(np.float32),
    )
    denom = _register_op("ANT_CF_DENOM", denom_spec)

    # out = Src1 / Src0 (approx): y0 = bitcast(~Src0)*c0 (seed ~6%), one
    # Newton step folded with the numerator multiply:
    #   out = (y0*Src1) * (c1 - Src0*y0)        -- 6 ALU stages.
    _y0 = Bin(AluOp.BITWISE_NOT, Src0, Src0) * C0

    def _rm_ref(in0, in1, s0, s1, imm2):
        y0 = (~in0.view(np.int32)).view(np.float32) * np.float32(s0)
        return ((y0 * in1) * (s1 - in0 * y0)).astype(np.float32)

    rm_spec = Spec(body=(_y0 * Src1) * (C1 - Src0 * _y0), reference=_rm_ref)
    recipmul = _register_op("ANT_CF_RECIPMUL", rm_spec)
    return denom, recipmul


def build_module():
    DENOM_OP, RECIPMUL_OP = _get_ops()
    f32 = mybir.dt.float32
    f16 = mybir.dt.float16
    TT = mybir.AluOpType
    SIG = mybir.ActivationFunctionType.Sigmoid
    SQU = mybir.ActivationFunctionType.Square
    CPY = mybir.ActivationFunctionType.Copy

    nc = bacc.Bacc("TRN2", target_bir_lowering=False, debug=False,
                   enable_asserts=False, num_devices=NCORES)

    xt_d = nc.dram_tensor("xt", [BPC, NFB, P, S], f16, kind="ExternalInput").ap()
    wt_d = nc.dram_tensor("wt", [K, NFB, P, F], f16, kind="ExternalInput").ap()
    cb_d = nc.dram_tensor("cb", [F, 1], f32, kind="ExternalInput").ap()
    yt_d = nc.dram_tensor("yt", [BPC, NFB, P, L], f16, kind="ExternalOutput").ap()

    with tile.TileContext(nc) as tc, ExitStack() as ctx:
        wpool = ctx.enter_context(tc.tile_pool(name="w", bufs=1))
        xpool = ctx.enter_context(tc.tile_pool(name="x", bufs=2))
        tpool = ctx.enter_context(tc.tile_pool(name="t", bufs=3))
        qpool = ctx.enter_context(tc.tile_pool(name="q", bufs=2))
        opool = ctx.enter_context(tc.tile_pool(name="o", bufs=2))
        ppool = ctx.enter_context(tc.tile_pool(name="p", bufs=2, space="PSUM"))

        def xh_dma(b):
            t = xpool.tile([P, NFB * S], f16, tag="xh", name=f"xh{b}")
            for ic in range(NFB):
                nc.sync.dma_start(t[:, ic * S:(ic + 1) * S], xt_d[b, ic])
            return t

        # batch 0's x is on the DMA critical path for the first matmul:
        # issue it ahead of the 1MB of weight transfers.
        xh_next = xh_dma(0)

        # weights + bias: loaded once, live forever
        w_sb = []
        for k in range(K):
            row = []
            for ic in range(NFB):
                t = wpool.tile([P, F], f16, tag=f"w{k}{ic}")
                nc.sync.dma_start(t[:], wt_d[k, ic])
                row.append(t)
            w_sb.append(row)
        bias_sb = wpool.tile([P, NFB], f32, tag="bias")
        nc.sync.dma_start(
            bias_sb[:], cb_d.rearrange("(ob p) one -> p (ob one)", p=P))

        for b in range(BPC):
            # x^T, both feature blocks side by side: [128, 2048] fp16
            xh = xh_next
            if b + 1 < BPC:
                xh_next = xh_dma(b + 1)
            # shifted fp16 copy (odd windows stay 4B-aligned in DVE 2x mode)
            xho = xpool.tile([P, NFB * S], f16, tag="xho")
            nc.scalar.activation(xho[:, :NFB * S - 1], xh[:, 1:NFB * S], CPY)

            # conv -> tau, per output-feature block; 4 PSUM tiles per batch.
            # Tile-major accumulation: each PSUM tile's 16 matmuls complete
            # before the next tile starts, so sigmoid drains tile (ob,li)
            # while the PE works on the next one (shorter pipeline fill).
            W2 = NFB * S
            tau = tpool.tile([P, W2], f16, tag="tau")
            # li-major so both feature blocks of the first l-half finish
            # first: batch 0's chain can then start on that half early.
            for li, l0 in enumerate((0, L - LT)):
                for ob in range(NFB):
                    ps = ppool.tile([P, LT], f32, tag=f"ps{ob}{li}",
                                    name=f"ps{ob}{li}_{b}")
                    for i, (ic, k) in enumerate(
                            (ic, k) for ic in range(NFB) for k in range(K)):
                        nc.tensor.matmul(
                            ps[:],
                            w_sb[k][ic][:, ob * P:(ob + 1) * P],
                            xh[:, ic * S + l0 + k: ic * S + l0 + k + LT],
                            start=(i == 0), stop=(i == NFB * K - 1),
                        )
                    nc.scalar.activation(
                        tau[:, ob * S + l0: ob * S + l0 + LT],
                        ps[:], SIG,
                        bias=bias_sb[:, ob:ob + 1], scale=1.0)
            t2 = tpool.tile([P, W2], f16, tag="t2")
            t4 = tpool.tile([P, W2], f16, tag="t4")
            nh = qpool.tile([P, W2], f16, tag="nh")

            def emit_chain(lo, hi):
                def pair(t, off=0):
                    return (t[:].rearrange("p (c n) -> p c n", c=2)
                            [:, :, off + lo: off + hi])

                nc.scalar.activation(pair(t2), pair(tau), SQU)
                nc.scalar.activation(pair(t4), pair(t2), SQU)
                th, t2p, t4p = pair(tau), pair(t2), pair(t4)

                def weven(i):
                    return pair(xh, i)

                def wodd(i):  # i odd; shifted copy at i-1 keeps alignment
                    return pair(xho, i - 1)

                # numerator chain, all fp16 2x-mode on DVE
                u0 = qpool.tile([P, W2], f16, tag="u")
                nc.vector.tensor_tensor(pair(u0), th, weven(0), TT.mult)
                q0 = qpool.tile([P, W2], f16, tag="q0")
                nc.vector.tensor_tensor(pair(q0), pair(u0), wodd(1), TT.add)
                m0 = qpool.tile([P, W2], f16, tag="m")
                nc.vector.tensor_tensor(pair(m0), pair(q0), t2p, TT.mult)

                u1 = qpool.tile([P, W2], f16, tag="u")
                nc.vector.tensor_tensor(pair(u1), th, weven(2), TT.mult)
                q1 = qpool.tile([P, W2], f16, tag="q1")
                nc.vector.tensor_tensor(pair(q1), pair(u1), wodd(3), TT.add)
                h0 = qpool.tile([P, W2], f16, tag="hh")
                nc.vector.tensor_tensor(pair(h0), pair(m0), pair(q1), TT.add)
                m1 = qpool.tile([P, W2], f16, tag="m")
                nc.vector.tensor_tensor(pair(m1), pair(h0), t4p, TT.mult)

                u2 = qpool.tile([P, W2], f16, tag="u")
                nc.vector.tensor_tensor(pair(u2), th, weven(4), TT.mult)
                q2 = qpool.tile([P, W2], f16, tag="q2")
                nc.vector.tensor_tensor(pair(q2), pair(u2), wodd(5), TT.add)
                h1 = qpool.tile([P, W2], f16, tag="hh")
                nc.vector.tensor_tensor(pair(h1), pair(q2), t2p, TT.mult)

                u3 = qpool.tile([P, W2], f16, tag="u")
                nc.vector.tensor_tensor(pair(u3), th, weven(6), TT.mult)
                q3 = qpool.tile([P, W2], f16, tag="q3")
                nc.vector.tensor_tensor(pair(q3), pair(u3), wodd(7), TT.add)
                h2 = qpool.tile([P, W2], f16, tag="h2")
                nc.vector.tensor_tensor(pair(h2), pair(h1), pair(q3), TT.add)

                nc.vector.tensor_tensor(pair(nh), pair(m1), pair(h2), TT.add)

            if b == 0:
                # pipeline fill: run the first l-half as soon as its two
                # sigmoid tiles land, overlapping the second half's conv
                emit_chain(0, LT)
                emit_chain(LT, LE)
            else:
                emit_chain(0, LE)

            # denominator + fused reciprocal*numerator (fp16 out, final)
            d = opool.tile([P, W2], f32, tag="d")
            nc.vector._custom_dve(DENOM_OP, out=d[:], in0=tau[:])
            oh = opool.tile([P, W2], f16, tag="oh")
            nc.vector._custom_dve(RECIPMUL_OP, out=oh[:], in0=d[:],
                                  in1=nh[:], s0=RM_C0, s1=RM_C1)
            for ob in range(NFB):
                nc.sync.dma_start(yt_d[b, ob], oh[:, ob * S: ob * S + L])

    nc.compile()
    return nc


_NC = None


def _get_nc():
    global _NC
    if _NC is None:
        _NC = build_module()
    return _NC


def prep_inputs(x, conv_w, conv_b):
    xt = np.ascontiguousarray(
        x.transpose(0, 2, 1)).astype(np.float16)
    xt = xt.reshape(B, NFB, P, S)
    wt = np.ascontiguousarray(
        conv_w.transpose(2, 1, 0)).astype(np.float16)
    wt = wt.reshape(K, NFB, P, F)
    cb = np.ascontiguousarray(conv_b, dtype=np.float32).reshape(F, 1)
    return xt, wt, cb


def make_in_maps(x, conv_w, conv_b):
    xt, wt, cb = prep_inputs(x, conv_w, conv_b)
    return [
        {"xt": xt[c * BPC:(c + 1) * BPC], "wt": wt, "cb": cb}
        for c in range(NCORES)
    ]


def gather_output(results):
    out = np.empty((B, L, F), np.float32)
    for c in range(NCORES):
        yt = results[c]["yt"]  # [BPC, NFB, P, L] fp16
        out[c * BPC:(c + 1) * BPC] = (
            yt.transpose(0, 3, 1, 2).reshape(BPC, L, F).astype(np.float32))
    return out


def kernel(x, conv_w, conv_b):
    nc = _get_nc()
    in_maps = make_in_maps(x, conv_w, conv_b)
    res = run_bass_kernel_spmd(nc, in_maps, core_ids=list(range(NCORES)))
    return gather_output(res.results)
